# revision 2
# baseline (speedup 1.0000x reference)
"""Batched multi-head attention (32 heads, S=2048, D=128, fp32) on 8 Trainium2
NeuronCores. Head-parallel sharding: core i computes heads [4i, 4i+4)
independently (no collectives), takes full fp32 inputs, returns the full fp32
output.

v2 design (vs the v1 181us kernel):
  - Q and K are pre-transposed to [h, d, s] on the HOST, so QT/KT load into
    SBUF with a plain dense cast-DMA (fp32->fp16). No PE-transposes, no DVE
    PSUM->SBUF copies, no psum slot borrowing.
  - q-chunks of 512 (4 per head). Scores^T for one (sk, chunk) is a
    [128, 512] fp32 psum block = exactly 1 bank. One big 6-bank psum tile
    holds a rolling window of 6 such blocks (sk-instance m -> bank m%6).
  - exp runs as ONE activation instruction per QUAD (4 sk tiles): a strided
    AP over 4 of the 6 banks ([128, 2048] = 4x fewer, 4x bigger ACT
    instructions; ACT per-instr overhead ~220-430ns amortizes). Quad bank
    sets cycle {0123} {4501} {2345}; the wrapped set uses a negative-stride
    AP so elements stay in natural sk order. While exp reads 4 banks, the
    PE fills the other 2 with the next quad's QK -> rolling, no stall.
  - PV: pt slice [128 sk, 128 q] stationary, V_aug [128 sk, 129] moving
    (col 128 = ones -> softmax denominator rides along). 16 MMs per quad
    accumulate into a 2-bank po tile: slices 0-2 at offsets 0/129/258
    (bank 6), slice 3 at offset 512 (bank 7). start=True only on the first
    MM touching each bank (clears the whole bank), stop at sk==15.
  - Normalize per chunk: 4x (DVE reciprocal of the ones column +
    tensor_scalar_mul) -> [128, 512] fp32 out tile, one DMA store.
  - Software-pipelined: PV(quad j-1) is emitted after exp(quad j); the
    chunk's normalize after the next chunk's first exp. po reuse across
    chunks is covered by the exp latency.
"""

import os
import numpy as np

BH, S, D = 32, 2048, 128
N_CORES = 8
HPC = BH // N_CORES  # heads per core
SK = S // 128        # 16 key tiles per head
CHUNK = 512          # q-chunk
NCH = S // CHUNK     # 4 chunks per head
QPC = SK // 4        # 4 quads per chunk
SCALE = 1.0 / float(np.sqrt(D))
PO_OFF = (0, 129, 258, 512)  # po column offsets for the 4 q-subtiles

_CACHE = {}


def _install_ntff_hook():
    """Provide antenv.axon_hooks (absent in this container) so that
    run_bass_kernel_spmd(trace=True) can capture NTFF profiles."""
    import contextlib, ctypes, sys, types

    if "antenv.axon_hooks" in sys.modules:
        return
    so_path = "/opt/axon/libaxon_pjrt.so"
    hook = None
    try:
        lib = ctypes.CDLL(so_path)
        if hasattr(lib, "axon_start_nrt_profile"):
            lib.axon_start_nrt_profile.argtypes = [
                ctypes.POINTER(ctypes.c_int64),
                ctypes.c_size_t,
            ]
            lib.axon_start_nrt_profile.restype = ctypes.c_int64
            lib.axon_stop_nrt_profile.argtypes = [ctypes.c_char_p]
            lib.axon_stop_nrt_profile.restype = ctypes.c_int64

            @contextlib.contextmanager
            def _h(output_dir, device_ids):
                import jax

                jax.devices()
                if device_ids:
                    ids = (ctypes.c_int64 * len(device_ids))(*device_ids)
                    rc = lib.axon_start_nrt_profile(ids, len(device_ids))
                else:
                    rc = lib.axon_start_nrt_profile(None, 0)
                if rc != 0:
                    raise RuntimeError(f"axon_start_nrt_profile rc={rc}")
                try:
                    yield
                finally:
                    n = lib.axon_stop_nrt_profile(str(output_dir).encode())
                    print(f"ntff profile: {n} file(s) in {output_dir}")

            hook = _h
    except OSError:
        pass
    mod = types.ModuleType("antenv.axon_hooks")
    mod.get_axon_ntff_profile_hook = lambda: hook
    mod.set_axon_ntff_profile_hook = lambda h: None
    sys.modules["antenv.axon_hooks"] = mod


def _split_sync_waits(nc, maxw=1):
    """The walrus codegen in this container rejects instructions carrying more
    than `maxw` sync waits (Tile's scheduler can attach several). Move the
    excess waits onto same-engine nop instructions inserted just before."""
    from concourse import mybir

    n_split = 0
    for f in nc.m.functions:
        for bb in f.blocks:
            out = []
            for inst in bb.instructions:
                si = inst.sync_info
                if si is not None and si.on_wait and len(si.on_wait) > maxw:
                    waits = list(si.on_wait)
                    carriers, keep = waits[:-maxw], waits[-maxw:]
                    si.on_wait = keep
                    inst.sync_info = si
                    for i in range(0, len(carriers), maxw):
                        n_split += 1
                        nop = mybir.InstNoOp(
                            name=f"{inst.name}_wsplit{i}", ins=[], outs=[]
                        )
                        nop.engine = inst.engine
                        nop.sync_info = mybir.SyncInfo(
                            on_wait=carriers[i : i + maxw], on_update=[]
                        )
                        if hasattr(nc, "inst_map"):
                            nc.inst_map[nop.name] = nop
                        out.append(nop)
                out.append(inst)
            bb.instructions[:] = out
    return n_split


def _build():
    import concourse.bass as bass
    from concourse import mybir
    import concourse.tile as tile
    import bass_rust

    fp16 = mybir.dt.float16
    fp32 = mybir.dt.float32
    AF = mybir.ActivationFunctionType

    from concourse.vector_clock import ScopedClock

    class SlimExitTileContext(tile.TileContext):
        def _drain_and_barrier(self, tick_clock, wait_clock):
            nc = self.nc
            drain_inst = nc.sync.drain()
            wait_clock.add_sem_waits(
                drain_inst.ins, ScopedClock({None: tick_clock.global_clock})
            )
            nc.all_engine_barrier()
            assert self.sems is not None
            popped = nc._tile_sem_poison_stack.pop()
            assert popped is self._sem_poison
            nc.clear_and_free_semaphores(list(self.sems.allocated().values()))
            nc.all_engine_barrier(sem_only=True)

    nc = bass.Bass("TRN2", target_bir_lowering=False, debug=False)
    # q, k arrive HOST-pre-transposed to [h, d, s]; v in natural [h, s, d]
    q = nc.dram_tensor("q", [HPC, D, S], fp32, kind="ExternalInput").ap()
    k = nc.dram_tensor("k", [HPC, D, S], fp32, kind="ExternalInput").ap()
    v = nc.dram_tensor("v", [HPC, S, D], fp32, kind="ExternalInput").ap()
    o = nc.dram_tensor("o", [HPC, S, D], fp32, kind="ExternalOutput").ap()

    with SlimExitTileContext(nc) as tc:
        with (
            tc.tile_pool(name="qt", bufs=2) as qt_pool,
            tc.tile_pool(name="kt", bufs=2) as kt_pool,
            tc.tile_pool(name="vsb", bufs=2) as v_pool,
            tc.tile_pool(name="pt", bufs=3) as pt_pool,
            tc.tile_pool(name="sc", bufs=1, space="PSUM") as sc_pool,
            tc.tile_pool(name="po", bufs=1, space="PSUM") as po_pool,
            tc.tile_pool(name="outsb", bufs=3) as out_pool,
            tc.tile_pool(name="norm", bufs=8) as norm_pool,
        ):
            # one 6-bank psum tile: rolling window of 6 [128,512] score blocks
            sc = sc_pool.tile([128, 6 * 512], fp32, name="scores")
            sc_ap = sc[:]

            def sc_sub(col_off, dims):
                """AP into the scores tile at column offset with explicit
                free dims [[stride, size], ...] (elements)."""
                return bass_rust.AP(
                    sc_ap.tensor,
                    sc_ap.offset + col_off,
                    [list(sc_ap.ap[0])] + [list(d) for d in dims],
                )

            qts, kts, vsbs = {}, {}, {}

            def prep_head(h, piece):
                """Load piece (0-3) of head h's inputs: kt quarter, qt
                quarter (chunk), v quarter. piece 0 also allocates tiles
                and memsets vsb (ones column augmentation)."""
                if h >= HPC:
                    return
                if piece == 0:
                    qts[h] = qt_pool.tile([128, S], fp16, tag="qt", name=f"qt_{h}")
                    kts[h] = kt_pool.tile([128, S], fp16, tag="kt", name=f"kt_{h}")
                    vsbs[h] = v_pool.tile(
                        [128, SK * 129], fp16, tag="vsb", name=f"vsb_{h}"
                    )
                    nc.gpsimd.memset(vsbs[h][:], 1.0)
                cs = slice(piece * 512, (piece + 1) * 512)
                nc.gpsimd.dma_start(kts[h][:, cs], k[h, :, cs])
                nc.gpsimd.dma_start(qts[h][:, cs], q[h, :, cs])
                vv = vsbs[h][:].rearrange("p (t c) -> p t c", c=129)
                ts = slice(piece * 4, (piece + 1) * 4)
                rows = slice(piece * 512, (piece + 1) * 512)
                nc.gpsimd.dma_start(
                    vv[:, ts, 0:D], v[h, rows, :].rearrange("(t p) d -> p t d", p=128)
                )

            for piece in range(4):
                prep_head(0, piece)

            pending = None   # emit the deferred PV batch of the previous quad
            finalize = None  # emit the previous chunk's normalize + store
            J = 0            # global quad index (bank pattern = (4J) % 6)
            for h in range(HPC):
                qt, kt, vsb = qts[h], kts[h], vsbs[h]
                for qc in range(NCH):
                    prep_head(h + 1, qc)
                    qbase = qc * CHUNK
                    po = po_pool.tile(
                        [128, 1024], fp32, tag="po", name=f"po_{h}_{qc}"
                    )
                    for jj in range(QPC):
                        b0 = (4 * J) % 6
                        # --- QK: 4 matmuls, one per sk tile, each into its
                        # own psum bank (start clears that bank)
                        for i in range(4):
                            sk = jj * 4 + i
                            bank = (b0 + i) % 6
                            out_ap = sc_sub(bank * 512, [[1, 512]])
                            nc.tensor.matmul(
                                out_ap,
                                kt[:, sk * 128 : (sk + 1) * 128],
                                qt[:, qbase : qbase + 512],
                                start=True,
                                stop=True,
                                skip_group_check=True,
                            )
                        # --- exp over the whole quad: one ACT instruction,
                        # strided AP over 4 banks; wrapped set {4,5,0,1}
                        # uses a negative outer stride to keep sk order
                        pt = pt_pool.tile([128, 4 * 512], fp16, tag="pt")
                        if b0 == 0:
                            in_ap = sc_sub(0, [[512, 4], [1, 512]])
                            out_ap = pt[:].rearrange("p (b c) -> p b c", c=512)
                        elif b0 == 2:
                            in_ap = sc_sub(1024, [[512, 4], [1, 512]])
                            out_ap = pt[:].rearrange("p (b c) -> p b c", c=512)
                        else:  # b0 == 4: banks {4,5,0,1}
                            in_ap = sc_sub(
                                2048, [[-2048, 2], [512, 2], [1, 512]]
                            )
                            out_ap = pt[:].rearrange(
                                "p (a b c) -> p a b c", a=2, b=2, c=512
                            )
                        nc.scalar.activation(out_ap, in_ap, AF.Exp, scale=SCALE)
                        if pending is not None:
                            pending()
                            pending = None
                        if jj == 0 and finalize is not None:
                            finalize()
                            finalize = None

                        def emit_pv(jj=jj, pt=pt, po=po, vsb=vsb):
                            for i in range(4):
                                sk = jj * 4 + i
                                for sq in range(4):
                                    off = PO_OFF[sq]
                                    nc.tensor.matmul(
                                        po[:, off : off + 129],
                                        pt[:, i * 512 + sq * 128 : i * 512 + sq * 128 + 128],
                                        vsb[:, sk * 129 : (sk + 1) * 129],
                                        start=(sk == 0 and (sq == 0 or sq == 3)),
                                        stop=(sk == SK - 1),
                                        skip_group_check=True,
                                    )

                        pending = emit_pv
                        J += 1

                    def make_finalize(po=po, h=h, qbase=qbase):
                        def fin():
                            ob = out_pool.tile(
                                [128, CHUNK], fp32, tag="ob", name=f"ob_{h}_{qbase}"
                            )
                            for sq in range(4):
                                off = PO_OFF[sq]
                                r = norm_pool.tile(
                                    [128, 1], fp32, tag="r", name=f"r_{h}_{qbase}_{sq}"
                                )
                                nc.vector.reciprocal(
                                    r[:], po[:, off + D : off + D + 1]
                                )
                                nc.vector.tensor_scalar_mul(
                                    ob[:, sq * D : (sq + 1) * D],
                                    po[:, off : off + D],
                                    r[:],
                                )
                            nc.sync.dma_start(
                                o[h, qbase : qbase + CHUNK, :].rearrange(
                                    "(t p) d -> p t d", p=128
                                ),
                                ob[:].rearrange("p (t d) -> p t d", d=D),
                            )

                        return fin

                    finalize = make_finalize()
            pending()
            finalize()

    _split_sync_waits(nc, maxw=1)
    return nc


def _get_nc():
    if "nc" not in _CACHE:
        _install_ntff_hook()
        _CACHE["nc"] = _build()
    return _CACHE["nc"]


def run_sharded(query, key, value, trace=False, **trace_kwargs):
    """Run the 8-core SPMD kernel; returns (output [BH,S,D] fp32, results)."""
    from concourse.bass_utils import run_bass_kernel_spmd

    nc = _get_nc()
    query = np.asarray(query, dtype=np.float32)
    key = np.asarray(key, dtype=np.float32)
    value = np.ascontiguousarray(np.asarray(value, dtype=np.float32))
    # host-side layout prep: Q, K as [h, d, s] for direct transposed loads
    qT = np.ascontiguousarray(query.transpose(0, 2, 1))
    kT = np.ascontiguousarray(key.transpose(0, 2, 1))
    in_maps = [
        {
            "q": qT[c * HPC : (c + 1) * HPC],
            "k": kT[c * HPC : (c + 1) * HPC],
            "v": value[c * HPC : (c + 1) * HPC],
        }
        for c in range(N_CORES)
    ]
    res = run_bass_kernel_spmd(
        nc, in_maps, list(range(N_CORES)), trace=trace, **trace_kwargs
    )
    out = np.concatenate([r["o"] for r in res.results], axis=0)
    return out, res


def kernel(key, query, value):
    out, _ = run_sharded(query, key, value, trace=False)
    return out
